# revision 28
# baseline (speedup 1.0000x reference)
"""Trainium2 Bass kernel for nn_CrossModalAttentionFusion.

Math: the module's two MultiheadAttention blocks run with sequence length 1,
so the softmax over a length-1 axis is identically 1.0 and q/k never affect
the output: each MHA reduces to  out = (fused @ Wv.T + bv) @ Wo.T + bo.
Folding the affine chains (done host-side, on the ~1M-param weights only):

    S        = text + image                      (host, elementwise)
    fused    = S @ fuse_w.T + fuse_b
    out_text = S @ (Wt @ fuse_w).T + (Wt @ fuse_b + t_out_w @ t_bv + t_out_b)
               where Wt = t_out_w @ t_wv
    out_image analogously.

Device work: one [B,1024] @ [1024,3072] fp16 matmul (the three projections
concatenated along the output dim), sharded batch-parallel over 8 cores.

Queue layout (per core): the eight 768 KB W tiles stream on three DMA
queues (ACT k0/k2/k6, Pool k1/k3/k5, SP k4/k7 after m0's S tiles) so each
lands just before m0's k-loop consumes it; the bias follows on SP, landing
before the first PSUM drain ever reads it. S tiles stream on SP ahead of
the PE. W stays in whole [128,3072] tiles and each m-tile's S arrives as one
[128,1024] tile (stationaries are 128-col slices): on this hardware,
fewer/bigger DMAs and fewer distinct-tile waits beat finer chunking every
time it was measured. Each 512-col PSUM group is drained (DVE bias-add)
as it closes; each m-tile stores once (1.5 MB) except the last, which
drains and stores per group, ending on a 128-col group so the kernel
tail is one narrow drain plus one 64 KB store.
"""

import numpy as np

import concourse.bass as bass  # noqa: F401  (registers engine methods)
import concourse.mybir as mybir
import concourse.tile as tile
from concourse import bacc
from concourse.bass_utils import run_bass_kernel_spmd

D = 1024
B = 16384
NCORES = 8
BS = B // NCORES          # 2048 rows per core
PT = 128                  # partition tile
MT = BS // PT             # 16 m-tiles per core
KT = D // PT              # 8 k-tiles (contraction)
NOUT = 3 * D              # fused | out_text | out_image
NFREE = 512               # psum-bank-sized free chunk per matmul
NT = NOUT // NFREE        # 6

MM_DT = mybir.dt.float16   # fp16 in / fp32 psum accumulate
NP_DT = np.float16
ORDER = "auto"             # m=0: k outer (start after first W chunk); rest: n outer

_COMPILED = None
LAST_RESULTS = None       # BassKernelResults from the most recent run


def _build(repeat=1, timing=False, order=ORDER, mm_dt=None,
           nfree=NFREE, mtiles=MT, store="full"):
    """Build the per-core NEFF.

    timing=True swaps the big external tensors for Internal DRAM (no host
    transfer) and adds a `repeat` hardware loop over the whole batch so
    device time can be extracted by wall-clock differencing between two
    repeat counts.

    order: "nk" = n outer / k inner (sequential psum groups, drained as
           they close); "kn" = k outer / n inner (stationary reused).
    """
    MM_DT = mm_dt if mm_dt is not None else globals()["MM_DT"]
    NT = NOUT // nfree
    psum_bufs = (8 * 512) // nfree
    nc = bacc.Bacc("TRN2", target_bir_lowering=False, debug=False,
                   num_devices=NCORES)
    if timing:
        st = nc.dram_tensor("st", [PT, KT * PT], MM_DT, kind="ExternalInput")
        w = nc.dram_tensor("w", [KT, PT, NOUT], MM_DT)
        bi = nc.dram_tensor("bi", [PT, NOUT], mybir.dt.float32)
        out = nc.dram_tensor("out", [BS, NOUT], mybir.dt.float32)
        tok = nc.dram_tensor("tok", [1, 1], mybir.dt.float32,
                             kind="ExternalOutput")
    else:
        st = nc.dram_tensor("st", [MT, PT, KT * PT], MM_DT,
                            kind="ExternalInput")
        w = nc.dram_tensor("w", [KT, PT, NOUT], MM_DT,
                           kind="ExternalInput")
        bi = nc.dram_tensor("bi", [PT, NOUT], mybir.dt.float32,
                            kind="ExternalInput")
        out = nc.dram_tensor("out", [BS, NOUT], mybir.dt.float32,
                             kind="ExternalOutput")
        tok = None

    with tile.TileContext(nc) as tc:
        with (
            tc.tile_pool(name="wpool", bufs=1) as wpool,
            tc.tile_pool(name="spool", bufs=8) as spool,
            tc.tile_pool(name="opool", bufs=4) as opool,
            tc.tile_pool(name="ppool", bufs=psum_bufs, space="PSUM") as ppool,
        ):
            # m0 consumes one 768 KB W tile every ~1.28us but one DMA queue
            # delivers one every ~2.4-3us, so W streams on three queues
            # arranged so tile k lands just before m0's k-loop needs it:
            # ACT gets k0,k2,k6; Pool k1,k3,k5; SP k4,k7 right after m0's
            # S tiles (whole-tile W, not 512-col chunks: one DMA semaphore
            # per k keeps the PE's per-matmul wait bookkeeping cheap).
            wts = []
            for k in range(KT):
                wt = wpool.tile([PT, NOUT], MM_DT, name=f"wt{k}",
                                tag=f"wt{k}")
                wts.append(wt)
            for k in (0, 2, 6):
                nc.scalar.dma_start(wts[k][:], w[k])
            for k in (1, 3, 5):
                nc.gpsimd.dma_start(wts[k][:], w[k])
            sp_w = [4, 7]  # issued in body at m==0 after S tiles
            # Bias rides the SP queue after m0's S tiles and k4/k7 (lands
            # ~11.4us, first drain needs it ~13us); SP has the slack.
            bt = wpool.tile([PT, NOUT], mybir.dt.float32, name="bt", tag="bt")
            bias_loaded = [False]
            if timing and repeat > 1:
                # One-time loads must stay out of the hardware repeat loop:
                # an in-loop re-DMA of a W tile makes every iteration's m0
                # wait on the previous iteration's last reader of that tile.
                for k in sp_w:
                    nc.sync.dma_start(wts[k][:], w[k])
                sp_w.clear()
                nc.sync.dma_start(bt[:], bi[:])
                bias_loaded[0] = True

            def body():
                for m in range(mtiles):
                    s_t = spool.tile([PT, KT * PT], MM_DT,
                                     name=f"s{m}", tag="s")
                    nc.sync.dma_start(s_t[:], st[:] if timing else st[m])
                    sts = [s_t[:, k * PT:(k + 1) * PT] for k in range(KT)]
                    if m == 0 and sp_w:
                        for k in sp_w:
                            nc.sync.dma_start(wts[k][:], w[k])
                        sp_w.clear()
                    if not bias_loaded[0]:
                        # must precede m0's drains in program order: a later
                        # issue would turn the drain's bt read into a read of
                        # the uninitialized tile (WAR, no data dependency)
                        nc.sync.dma_start(bt[:], bi[:])
                        bias_loaded[0] = True
                    ot = opool.tile([PT, NOUT], mybir.dt.float32,
                                    name=f"o{m}", tag="o")
                    last_m = m == mtiles - 1
                    # the last m-tile ends with a 128-col group so the
                    # post-last-matmul chain (drain + store) is ~4x shorter
                    if last_m and nfree > 128:
                        groups = ([(i * nfree, nfree) for i in range(NT - 1)]
                                  + [((NT - 1) * nfree, nfree - 128),
                                     (NOUT - 128, 128)])
                    else:
                        groups = [(i * nfree, nfree) for i in range(NT)]
                    pts = [ppool.tile([PT, nfree], mybir.dt.float32,
                                      name=f"p{m}_{g}", tag="p")
                           for g in range(len(groups))]

                    def drain(g):
                        off, wd = groups[g]
                        gsl = slice(off, off + wd)
                        nc.vector.tensor_add(ot[:, gsl], pts[g][:, :wd],
                                             bt[:, gsl])
                        if store == "chunk" or last_m:
                            # per-group on the last tile: the kernel tail is
                            # one narrow drain + one 64 KB store
                            ssl = gsl
                        elif store == "half" and g in (NT // 2 - 1, NT - 1):
                            ssl = (slice(0, NOUT // 2) if g == NT // 2 - 1
                                   else slice(NOUT // 2, NOUT))
                        elif store == "full" and g == len(groups) - 1:
                            ssl = slice(0, NOUT)
                        else:
                            ssl = None
                        if ssl is not None:
                            nc.scalar.dma_start(out[m * PT:(m + 1) * PT, ssl],
                                                ot[:, ssl])

                    m_order = order
                    if order == "auto":
                        m_order = "kn" if m == 0 else "nk"
                    if m_order == "nk":
                        for g, (off, wd) in enumerate(groups):
                            for k in range(KT):
                                nc.tensor.matmul(
                                    pts[g][:, :wd], sts[k],
                                    wts[k][:, off:off + wd],
                                    start=(k == 0), stop=(k == KT - 1),
                                )
                            drain(g)
                    else:  # "kn"
                        for k in range(KT):
                            for g, (off, wd) in enumerate(groups):
                                nc.tensor.matmul(
                                    pts[g][:, :wd], sts[k],
                                    wts[k][:, off:off + wd],
                                    start=(k == 0), stop=(k == KT - 1),
                                )
                        for g in range(len(groups)):
                            drain(g)

            if repeat > 1:
                with tc.For_i(0, repeat, 1,
                              hint_engines=(mybir.EngineType.PE,)):
                    body()
            else:
                body()
            if tok is not None:
                tk = wpool.tile([1, 1], mybir.dt.float32, name="tk", tag="tk")
                nc.gpsimd.memset(tk[:], 1.0)
                nc.sync.dma_start(tok[:], tk[:])

    nc.compile()
    return nc


def _fold_params(fuse_w, fuse_b, t_in_w, t_in_b, t_out_w, t_out_b,
                 i_in_w, i_in_b, i_out_w, i_out_b):
    """Host-side weight folding (float64). Returns W_all [D, 3D], bias_all [3D]."""
    f8 = np.float64
    fuse_w8, fuse_b8 = fuse_w.astype(f8), fuse_b.astype(f8)

    def fold(in_w, in_b, out_w, out_b):
        wv = in_w[2 * D:3 * D].astype(f8)
        bv = in_b[2 * D:3 * D].astype(f8)
        Wm = out_w.astype(f8) @ wv                    # fused -> out
        bm = out_w.astype(f8) @ bv + out_b.astype(f8)
        W2 = Wm @ fuse_w8                             # S -> out
        b2 = Wm @ fuse_b8 + bm
        return W2, b2

    Wt2, bias_t = fold(t_in_w, t_in_b, t_out_w, t_out_b)
    Wi2, bias_i = fold(i_in_w, i_in_b, i_out_w, i_out_b)

    W_all = np.empty((D, NOUT), np.float32)
    W_all[:, 0:D] = fuse_w8.T
    W_all[:, D:2 * D] = Wt2.T
    W_all[:, 2 * D:3 * D] = Wi2.T
    bias_all = np.empty(NOUT, np.float32)
    bias_all[0:D] = fuse_b
    bias_all[D:2 * D] = bias_t
    bias_all[2 * D:3 * D] = bias_i
    return W_all, bias_all


def kernel(text_feat, image_feat, fuse_w, fuse_b,
           t_in_w, t_in_b, t_out_w, t_out_b,
           i_in_w, i_in_b, i_out_w, i_out_b):
    global _COMPILED, LAST_RESULTS
    text_feat = np.asarray(text_feat, np.float32)
    image_feat = np.asarray(image_feat, np.float32)
    args = [np.asarray(a, np.float32) for a in
            (fuse_w, fuse_b, t_in_w, t_in_b, t_out_w, t_out_b,
             i_in_w, i_in_b, i_out_w, i_out_b)]
    W_all, bias_all = _fold_params(*args)

    S = text_feat + image_feat                        # (B, D)
    # Per-core pre-tiled S^T: st[m,k,p,q] = S[core*BS + m*128 + q, k*128 + p]
    in_maps = []
    w_arr = np.ascontiguousarray(W_all.reshape(KT, PT, NOUT).astype(NP_DT))
    bi_arr = np.ascontiguousarray(np.broadcast_to(bias_all, (PT, NOUT)))
    for c in range(NCORES):
        Sc = S[c * BS:(c + 1) * BS]                   # (BS, D)
        stc = np.ascontiguousarray(
            Sc.reshape(MT, PT, KT, PT).transpose(0, 3, 2, 1)
            .reshape(MT, PT, KT * PT).astype(NP_DT))
        in_maps.append({"st": stc, "w": w_arr, "bi": bi_arr})

    if _COMPILED is None:
        _COMPILED = _build()

    LAST_RESULTS = run_bass_kernel_spmd(
        _COMPILED, in_maps, core_ids=list(range(NCORES)))
    outs = np.concatenate([r["out"] for r in LAST_RESULTS.results], axis=0)

    fused = outs[:, 0:D]
    out_text = outs[:, D:2 * D]
    out_image = outs[:, 2 * D:3 * D]
    return (out_text, out_image, fused)
